# revision 40
# baseline (speedup 1.0000x reference)
"""Multi-head attention (B=2, T=4096, D=512, H=8) on 8 Trainium2 cores.

Sharding: core i handles batch b=i//4, query rows q0=(i%4)*1024 .. q0+1024,
all 8 heads (full K/V of its batch computed on-core; no collectives).
Host pre-transposes x and weights (bf16) and rolls x along T per core so
each core's query block sits at columns 0:1024.

v2 pipeline (exp-bound design):
- Projections in bf16 (x, W all bf16; f32 PSUM accumulation).
- Scores via fp8e4(e4m3) DoubleRow matmuls: K^T/Q^T drained to fp8 with a
  zeroed second k-tile slot, so one DR matmul contracts the full d_k=64 at
  0.5 cycles/row.  exp applies the 1/sqrt(d_k)=1/8 scale for free.
- exp on the Activation engine in 3-bank (1536-col) PSUM chunks, double
  buffered through a 6-bank ring; output bf16 `at` tiles.
- AV with swapped operands (stationary=at [128k,128q], moving=V|1 [128k,65])
  accumulating [128q, 65] in a single PSUM bank; per-partition rowsum ->
  reciprocal_approx_fast + normalize to bf16 (DVE), then DMA-transpose
  [128q,128d] -> acat [128d, q].  Output projection partials (pairs 0-2)
  pre-accumulated into SBUF so the tail is one matmul + add per block.
- bv is folded into bo on the host (bo' = bo + bv @ Wo.T), bq/bk folded
  into the fp8 drains.
- Output projection in bf16 + f32r bias matmul, f32 result.
"""
import sys
sys.path.insert(0, "/opt/trn_rl_repo")

import numpy as np
import ml_dtypes
import concourse.bacc as bacc
import concourse.mybir as mybir
import concourse.tile as tile
from concourse.bass_utils import run_bass_kernel_spmd

F32 = mybir.dt.float32
F32R = mybir.dt.float32r
BF16 = mybir.dt.bfloat16
F8 = mybir.dt.float8e4
AF = mybir.ActivationFunctionType
ADD = mybir.AluOpType.add
MULT = mybir.AluOpType.mult
DR = mybir.MatmulPerfMode.DoubleRow

B, T, C = 2, 4096, 512
H, DK = 8, 64
TQ = 1024          # queries per core
NP = 4             # head pairs
KT = T // 128      # 32 k-tiles
CT = C // 128      # 4 contraction tiles
NPH = 2 * H        # 16 phases (head, q-half)

_cache = {}


def _build():
    nc = bacc.Bacc("TRN2")
    xbT = nc.declare_dram_parameter("xbT", [C, T], BF16, isOutput=False)
    wqT = nc.declare_dram_parameter("wqT", [C, C], BF16, isOutput=False)
    wkT = nc.declare_dram_parameter("wkT", [C, C], BF16, isOutput=False)
    wvT = nc.declare_dram_parameter("wvT", [C, C], BF16, isOutput=False)
    woT = nc.declare_dram_parameter("woT", [C, C], BF16, isOutput=False)
    # bias[:, 0, p] = bq slice, bias[:, 1, p] = bk slice
    bias = nc.declare_dram_parameter("bias", [128, 2, NP], F32, isOutput=False)
    bof = nc.declare_dram_parameter("bof", [1, C], F32R, isOutput=False)
    ones1 = nc.declare_dram_parameter("ones1", [1, 128], F32R, isOutput=False)
    out = nc.declare_dram_parameter("out", [TQ, C], F32, isOutput=True)

    with tile.TileContext(nc) as tc:
        with (
            tc.tile_pool(name="big", bufs=1) as bpool,
            tc.tile_pool(name="v2", bufs=2) as v2pool,
            tc.tile_pool(name="rc", bufs=4) as rpool,
            tc.tile_pool(name="ot", bufs=4) as opool,
            tc.tile_pool(name="ring", bufs=2, space="PSUM") as ring,
            tc.tile_pool(name="avp", bufs=1, space="PSUM") as avp,
            tc.tile_pool(name="prj", bufs=1, space="PSUM") as prj,
        ):
            # ---- static SBUF tiles ----
            xT = bpool.tile([128, CT, T], BF16, tag="xT")        # 32KB/part
            woTs = bpool.tile([128, CT, C], BF16, tag="woT")     # 4KB
            biasS = bpool.tile([128, 2, NP], F32, tag="bias")
            onesO = bpool.tile([65, 128], F32R, tag="ones")
            boS = bpool.tile([65, C], F32R, tag="bo")
            # fp8 K^T/Q^T, double-buffered by pair parity; [:,1,:] stays 0
            kf8a = bpool.tile([128, 2, T], F8, tag="kf8a")
            kf8b = bpool.tile([128, 2, T], F8, tag="kf8b")
            qf8a = bpool.tile([128, 2, TQ], F8, tag="qf8a")
            qf8b = bpool.tile([128, 2, TQ], F8, tag="qf8b")
            kf8 = [kf8a, kf8b]
            qf8 = [qf8a, qf8b]
            # exp output, double-buffered by phase parity
            at0 = bpool.tile([128, KT, 512], BF16, tag="at0")    # 32KB
            at1 = bpool.tile([128, KT, 512], BF16, tag="at1")    # 32KB
            at = [at0, at1]
            # normalized AV, [q, d-pair]; double-buffered by pair parity
            avn0 = bpool.tile([128, 8, 128], BF16, tag="avn0")
            avn1 = bpool.tile([128, 8, 128], BF16, tag="avn1")
            avn = [avn0, avn1]
            acat = bpool.tile([128, NP, TQ], BF16, tag="acat")   # 8KB
            oacc = bpool.tile([128, 8, C], F32, tag="oacc")      # 16KB
            wkS = bpool.tile([128, CT, C], BF16, tag="wkS")
            wqS = bpool.tile([128, CT, C], BF16, tag="wqS")
            wvS = bpool.tile([128, CT, C], BF16, tag="wvS")

            # ---- prologue DMAs ----
            xv = xbT.rearrange("(ct p) t -> p ct t", p=128)
            wkv = wkT.rearrange("(ct p) c -> p ct c", p=128)
            wqv = wqT.rearrange("(ct p) c -> p ct c", p=128)
            wvv = wvT.rearrange("(ct p) c -> p ct c", p=128)
            nc.sync.dma_start(wkS[:], wkv[:])
            nc.sync.dma_start(xT[:, :, 0:512], xv[:, :, 0:512])
            nc.sync.dma_start(wqS[:], wqv[:])
            nc.sync.dma_start(biasS[:], bias[:])
            nc.sync.dma_start(xT[:, :, 512:1024], xv[:, :, 512:1024])
            nc.sync.dma_start(wvS[:], wvv[:])
            for tch in range(1, 4):
                nc.sync.dma_start(
                    xT[:, :, tch * 1024:(tch + 1) * 1024],
                    xv[:, :, tch * 1024:(tch + 1) * 1024])
            nc.sync.dma_start(onesO[64:65, :],
                              ones1.rearrange("(o a) b -> o a b", o=1))
            nc.sync.dma_start(boS[64:65, :], bof.rearrange("(o a) b -> o a b", o=1))
            nc.sync.dma_start(woTs[:], woT.rearrange("(ct p) c -> p ct c", p=128))
            dz = bpool.tile([64, 2, 512], F8, tag="dz")
            nc.vector.memset(dz[:], 0.0)
            nc.vector.memset(kf8[0][:, 1, 0:1536], 0.0)
            nc.vector.memset(qf8[0][:, 1, :], 0.0)
            nc.gpsimd.memset(kf8[0][:, 1, 1536:T], 0.0)
            nc.gpsimd.memset(kf8[1][:, 1, :], 0.0)
            nc.gpsimd.memset(qf8[1][:, 1, :], 0.0)

            # ---- helper emitters (each returns a list of zero-arg thunks) --

            def k_piece(p, piece):
                # K^T cols piece*512:(piece+1)*512 -> kf8[p%2][:, 0, ...]
                pp = prj.tile([128, 512], F32, tag="prj")
                for ct in range(CT):
                    nc.tensor.matmul(
                        pp[:], wkS[:, ct, p * 128:(p + 1) * 128],
                        xT[:, ct, piece * 512:(piece + 1) * 512],
                        start=(ct == 0), stop=(ct == CT - 1))
                nc.vector.tensor_scalar_add(
                    kf8[p % 2][:, 0, piece * 512:(piece + 1) * 512], pp[:],
                    biasS[:, 1, p:p + 1])

            def q_piece(p, piece):
                pp = prj.tile([128, 512], F32, tag="prj")
                for ct in range(CT):
                    nc.tensor.matmul(
                        pp[:], wqS[:, ct, p * 128:(p + 1) * 128],
                        xT[:, ct, piece * 512:(piece + 1) * 512],
                        start=(ct == 0), stop=(ct == CT - 1))
                nc.vector.tensor_scalar_add(
                    qf8[p % 2][:, 0, piece * 512:(piece + 1) * 512], pp[:],
                    biasS[:, 0, p:p + 1])

            def v_piece(v2p, pg, j, pool=None):
                # V rows for k-tiles j, j+1 as separate exact-cover groups.
                for jj in range(2):
                    pv = (pool or prj).tile([128, 512], F32,
                                            tag="av" if pool is avp else "prj")
                    for ct in range(CT):
                        nc.tensor.matmul(
                            pv[:, 0:256],
                            xT[:, ct, (j + jj) * 128:(j + jj + 1) * 128],
                            wvS[:, ct, pg * 256:(pg + 1) * 256],
                            start=(ct == 0), stop=(ct == CT - 1))
                    nc.vector.tensor_copy(
                        v2p[:, j + jj, :, 0:64],
                        pv[:, 0:256].rearrange("p (h b) -> p h b", b=64))

            def new_v2p():
                v2p = v2pool.tile([128, KT, 4, 65], BF16, tag="v2p")
                nc.gpsimd.memset(v2p[:, :, :, 64], 1.0)
                return v2p

            def av_group(ph, qb, v2p, av_ap=None):
                # AV for phase ph (= head h, q-half), query block qb (0..3)
                h, half = ph // 2, ph % 2
                hb = ph % 2  # at buffer parity
                d0 = (h % 2) * 64
                pb = (h // 2) % 2
                qbg = half * 4 + qb
                if av_ap is None:
                    av_t = avp.tile([128, 512], F32, tag="av")
                else:
                    av_t = av_ap
                for kt in range(KT):
                    nc.tensor.matmul(
                        av_t[:, 0:65],
                        at[hb][:, kt, qb * 128:(qb + 1) * 128],
                        v2p[:, kt, h % 4, :],
                        start=(kt == 0), stop=(kt == KT - 1))
                rec = rpool.tile([128, 1], F32, tag="rec")
                nc.vector.reciprocal_approx_fast(rec[:], av_t[:, 64:65])
                nc.vector.tensor_scalar(
                    avn[pb][:, qbg, d0:d0 + 64], av_t[:, 0:64],
                    rec[:], None, MULT)

            def o_piece(qt, po=None):
                if po is None:
                    po = prj.tile([128, 512], F32, tag="prj")
                nc.tensor.matmul(
                    po[:], acat[:, 3, qt * 128:(qt + 1) * 128],
                    woTs[:, 3, :], start=True, stop=True)
                ot = opool.tile([128, 512], F32, tag="ot")
                nc.vector.tensor_tensor(out=ot[:], in0=po[:],
                                        in1=oacc[:, qt, :], op=ADD)
                nc.sync.dma_start(out[qt * 128:(qt + 1) * 128, :], ot[:])

            # ---- prologue compute: pair-0 K/Q on ring slots (batched drains) --

            def ring_kq(groups):
                """groups: list of ('k'|'q', p, piece). One ring tile, one
                matmul group per bank, batched drains per contiguous run."""
                rt = ring.tile([128, 1536], F32, tag="ring")
                for g, (kind, p, piece) in enumerate(groups):
                    w = wkS if kind == "k" else wqS
                    for ct in range(CT):
                        nc.tensor.matmul(
                            rt[:, g * 512:(g + 1) * 512],
                            w[:, ct, p * 128:(p + 1) * 128],
                            xT[:, ct, piece * 512:(piece + 1) * 512],
                            start=(ct == 0), stop=(ct == CT - 1))
                # batched drains over contiguous same-kind runs
                g = 0
                while g < len(groups):
                    kind, p, piece = groups[g]
                    g2 = g
                    while (g2 + 1 < len(groups)
                           and groups[g2 + 1][0] == kind
                           and groups[g2 + 1][2] == groups[g2][2] + 1):
                        g2 += 1
                    dst = kf8[p % 2] if kind == "k" else qf8[p % 2]
                    bcol = 1 if kind == "k" else 0
                    nc.vector.tensor_scalar_add(
                        dst[:, 0, piece * 512:piece * 512 + (g2 - g + 1) * 512],
                        rt[:, g * 512:(g2 + 1) * 512],
                        biasS[:, bcol, p:p + 1])
                    g = g2 + 1

            # PE p-state warm-up on zeros while x loads
            wup = avp.tile([128, 512], F32, tag="av")
            for i in range(14):
                nc.tensor.matmul(wup[:], dz[:, :, 0:128], dz[:],
                                 start=True, stop=True, perf_mode=DR,
                                 tile_position=(0, 0))
            ring_kq([("k", 0, 0)])
            q_piece(0, 0)
            v2p_cur = new_v2p()

            # ---- main pipeline over 16 phases ----
            state = {"v2p": v2p_cur, "v2p_next": None, "pending": [],
                     "o_pending": []}

            def phase_background(ph):
                """Thunks to interleave into phase ph's chunk stream."""
                thunks = []
                h, half = ph // 2, ph % 2
                p = h // 2
                # deferred transposes/O-pieces from the previous phase first
                pend, state["pending"] = state["pending"], []
                thunks.extend(pend)
                # leftover pg0 V-pieces MUST precede phase-0's AV groups
                if ph == 1:
                    for j in range(28, KT, 2):
                        thunks.append(lambda j=j: v_piece(state["v2p"], 0, j))
                # AV of previous phase (+ deferred transpose & O-proj)
                if ph >= 1:
                    prev_h = (ph - 1) // 2
                    v2p_prev = (state["v2p_prev4"] if prev_h // 4 != h // 4
                                else state["v2p"])
                    for qb in range(4):
                        thunks.append(lambda ph=ph, qb=qb, v=v2p_prev:
                                      av_tr_o(ph - 1, qb, v))
                # projection prep for pair p+1
                slot = ph % 4
                if p + 1 < NP:
                    if slot == 2:
                        for piece in range(5):
                            thunks.append(lambda p=p, piece=piece:
                                          k_piece(p + 1, piece))
                    elif slot == 3:
                        for piece in range(5, 8):
                            thunks.append(lambda p=p, piece=piece:
                                          k_piece(p + 1, piece))
                        for piece in range(2):
                            thunks.append(lambda p=p, piece=piece:
                                          q_piece(p + 1, piece))
                # pair-0 K piece 7 early in phase 0
                if ph == 0:
                    thunks.append(lambda: ring_kq(
                        [("q", 0, 1), ("k", 0, 1), ("k", 0, 2)]))
                    thunks.append(lambda: ring_kq(
                        [("k", 0, 3), ("k", 0, 4), ("k", 0, 5)]))
                    thunks.append(lambda: k_piece(0, 6))
                    thunks.append(lambda: k_piece(0, 7))
                    for i, j in enumerate(range(0, 28, 2)):
                        thunks.append(lambda j=j, i=i: v_piece(
                            state["v2p"], 0, j,
                            pool=avp if i % 2 else prj))
                if 4 <= ph <= 7:
                    if ph == 4:
                        def mkv():
                            state["v2p_next"] = new_v2p()
                        thunks.append(mkv)
                    for j in range((ph - 4) * 8, (ph - 4) * 8 + 8, 2):
                        thunks.append(lambda j=j: v_piece(state["v2p_next"],
                                                          1, j))
                if ph == 13:
                    for qt in range(4):
                        thunks.append(lambda qt=qt: o_partial(qt))
                if ph == 14:
                    for qt in range(4, 8):
                        thunks.append(lambda qt=qt: o_partial(qt))
                return thunks

            def o_partial(qt):
                po = prj.tile([128, 512], F32, tag="prj")
                for r in range(3):
                    nc.tensor.matmul(
                        po[:], acat[:, r, qt * 128:(qt + 1) * 128],
                        woTs[:, r, :], start=(r == 0), stop=False)
                nc.tensor.matmul(po[:], onesO[64:65, :], boS[64:65, :],
                                 start=False, stop=True)
                nc.vector.tensor_copy(oacc[:, qt, :], po[:])

            def tr_o(p, qbg, po=None):
                nc.sync.dma_start_transpose(
                    acat[:, p, qbg * 128:(qbg + 1) * 128],
                    avn[p % 2][:, qbg, :])
                if p == NP - 1:
                    o_piece(qbg, po)

            def av_tr_o(ph, qb, v2p, av_ap=None, po=None, defer=True):
                """AV group + (for odd heads) transpose + (pair 3) O-proj."""
                av_group(ph, qb, v2p, av_ap)
                h, half = ph // 2, ph % 2
                if h % 2 == 1:
                    p = h // 2
                    qbg = half * 4 + qb
                    if p == NP - 1:
                        # last pair: transpose deferred, O-piece to epilogue
                        state["pending"].append(
                            lambda qbg=qbg: nc.sync.dma_start_transpose(
                                acat[:, 3, qbg * 128:(qbg + 1) * 128],
                                avn[1][:, qbg, :]))
                        state["o_pending"].append(qbg)
                    elif defer:
                        state["pending"].append(
                            lambda p=p, qbg=qbg, po=po: tr_o(p, qbg, po))
                    else:
                        tr_o(p, qbg, po)

            for ph in range(NPH):
                h, half = ph // 2, ph % 2
                if ph == 8:
                    state["v2p_prev4"] = state["v2p"]
                    state["v2p"] = state["v2p_next"]
                state.setdefault("v2p_prev4", state["v2p"])
                d0 = (h % 2) * 64
                kcur, qcur = kf8[h // 2 % 2], qf8[h // 2 % 2]
                bg = phase_background(ph)
                bgi = 0
                # 11 chunks: 10x3 kt + 1x2 kt
                for c in range(11):
                    n = 3 if c < 10 else 2
                    ring_t = ring.tile([128, 1536], F32, tag="ring")
                    for jj in range(n):
                        kt = 3 * c + jj
                        nc.tensor.matmul(
                            ring_t[:, jj * 512:(jj + 1) * 512],
                            kcur[d0:d0 + 64, :, kt * 128:(kt + 1) * 128],
                            qcur[d0:d0 + 64, :, half * 512:(half + 1) * 512],
                            start=True, stop=True, perf_mode=DR,
                            tile_position=(d0, 0))
                    nc.scalar.activation(
                        at[ph % 2][:, 3 * c:3 * c + n, :],
                        ring_t[:, 0:512 * n].rearrange("p (a b) -> p a b", b=512),
                        AF.Exp, scale=0.125)
                    # interleave background work: spread across chunks
                    n_bg = (len(bg) * (c + 1)) // 11 - (len(bg) * c) // 11
                    for _ in range(n_bg):
                        bg[bgi]()
                        bgi += 1
                assert bgi == len(bg)
                if ph == NPH - 1:
                    # flush pair-3 transposes first, then their O-pieces
                    pend, state["pending"] = state["pending"], []
                    for th in pend:
                        th()
                    for qt in state["o_pending"][:4]:
                        o_piece(qt)
                    state["o_pending"] = state["o_pending"][4:]

            # ---- epilogue: AV of phase 15 + pair-3 transposes + O-proj ----
            # ring banks are free: give every AV group and O-piece its own
            # bank and software-pipeline so no PE wait blocks later work.
            for th in state["pending"]:
                th()
            h15 = (NPH - 1) // 2
            v2p15 = state["v2p"]
            avA = avp.tile([128, 512], F32, tag="av")
            epA = ring.tile([128, 1536], F32, tag="ring")
            av_aps = [avA, epA[:, 0:512], epA[:, 512:1024], epA[:, 1024:1536]]
            for kt in range(KT):
                for qb in range(4):
                    nc.tensor.matmul(
                        av_aps[qb][:, 0:65],
                        at[(NPH - 1) % 2][:, kt, qb * 128:(qb + 1) * 128],
                        v2p15[:, kt, h15 % 4, :],
                        start=(kt == 0), stop=(kt == KT - 1))

            def norm_tr(qb, t):
                qbg = 4 + qb
                rec = rpool.tile([128, 1], F32, tag="rec")
                nc.vector.reciprocal_approx_fast(rec[:], t[:, 64:65])
                nc.vector.tensor_scalar(
                    avn[1][:, qbg, 64:128], t[:, 0:64], rec[:], None, MULT)
                nc.sync.dma_start_transpose(
                    acat[:, 3, qbg * 128:(qbg + 1) * 128],
                    avn[1][:, qbg, :])

            for qb in range(4):
                norm_tr(qb, av_aps[qb])
            epB = ring.tile([128, 1536], F32, tag="ring")
            avB = avp.tile([128, 512], F32, tag="av")
            for i, qt in enumerate([4, 5, 6, 7]):
                o_piece(qt, po=epB[:, i * 512:(i + 1) * 512]
                        if i < 3 else avB)

    nc.compile()
    return nc


def _prep_inputs(x, Wq, bq, Wk, bk, Wv, bv, Wo, bo):
    bf = ml_dtypes.bfloat16
    wqT = np.ascontiguousarray(Wq.T).astype(bf)
    wkT = np.ascontiguousarray(Wk.T).astype(bf)
    wvT = np.ascontiguousarray(Wv.T).astype(bf)
    woT = np.ascontiguousarray(Wo.T).astype(bf)
    bias = np.stack([
        bq.reshape(NP, 128).T,
        bk.reshape(NP, 128).T,
    ], axis=1).astype(np.float32)          # [128, 2, NP]
    bias = np.ascontiguousarray(bias)
    bof = np.ascontiguousarray(
        (bo.astype(np.float64) + bv.astype(np.float64) @ Wo.astype(np.float64).T)
        .reshape(1, C)).astype(np.float32)
    ones1 = np.ones((1, 128), np.float32)
    in_maps = []
    for i in range(8):
        b, q0 = i // 4, (i % 4) * TQ
        xbT = np.ascontiguousarray(np.roll(x[b].T, -q0, axis=1)).astype(bf)
        in_maps.append({
            "xbT": xbT, "wqT": wqT, "wkT": wkT, "wvT": wvT, "woT": woT,
            "bias": bias, "bof": bof, "ones1": ones1,
        })
    return in_maps


def kernel(x, Wq, bq, Wk, bk, Wv, bv, Wo, bo):
    x = np.asarray(x, np.float32)
    args = [np.asarray(a, np.float32) for a in
            (Wq, bq, Wk, bk, Wv, bv, Wo, bo)]
    if "nc" not in _cache:
        _cache["nc"] = _build()
    nc = _cache["nc"]
    in_maps = _prep_inputs(x, *args)
    res = run_bass_kernel_spmd(nc, in_maps, list(range(8)))
    outf = np.empty((B, T, C), np.float32)
    for i in range(8):
        b, q0 = i // 4, (i % 4) * TQ
        outf[b, q0:q0 + TQ, :] = res.results[i]["out"]
    return outf


# revision 41
# speedup vs baseline: 1.0129x; 1.0129x over previous
"""Multi-head attention (B=2, T=4096, D=512, H=8) on 8 Trainium2 cores.

Sharding: core i handles batch b=i//4, query rows q0=(i%4)*1024 .. q0+1024,
all 8 heads (full K/V of its batch computed on-core; no collectives).
Host pre-transposes x and weights (bf16) and rolls x along T per core so
each core's query block sits at columns 0:1024.

v2 pipeline (exp-bound design):
- Projections in bf16 (x, W all bf16; f32 PSUM accumulation).
- Scores via fp8e4(e4m3) DoubleRow matmuls: K^T/Q^T drained to fp8 with a
  zeroed second k-tile slot, so one DR matmul contracts the full d_k=64 at
  0.5 cycles/row.  exp applies the 1/sqrt(d_k)=1/8 scale for free.
- exp on the Activation engine in 3-bank (1536-col) PSUM chunks, double
  buffered through a 6-bank ring; output bf16 `at` tiles.
- AV with swapped operands (stationary=at [128k,128q], moving=V|1 [128k,65])
  accumulating [128q, 65] in a single PSUM bank; per-partition rowsum ->
  reciprocal_approx_fast + normalize to bf16 (DVE), then DMA-transpose
  [128q,128d] -> acat [128d, q].  Output projection partials (pairs 0-2)
  pre-accumulated into SBUF so the tail is one matmul + add per block.
- bv is folded into bo on the host (bo' = bo + bv @ Wo.T), bq/bk folded
  into the fp8 drains.
- Output projection in bf16 + f32r bias matmul, f32 result.
"""
import sys
sys.path.insert(0, "/opt/trn_rl_repo")

import numpy as np
import ml_dtypes
import concourse.bacc as bacc
import concourse.mybir as mybir
import concourse.tile as tile
from concourse.bass_utils import run_bass_kernel_spmd

F32 = mybir.dt.float32
F32R = mybir.dt.float32r
BF16 = mybir.dt.bfloat16
F8 = mybir.dt.float8e4
AF = mybir.ActivationFunctionType
ADD = mybir.AluOpType.add
MULT = mybir.AluOpType.mult
DR = mybir.MatmulPerfMode.DoubleRow

B, T, C = 2, 4096, 512
H, DK = 8, 64
TQ = 1024          # queries per core
NP = 4             # head pairs
KT = T // 128      # 32 k-tiles
CT = C // 128      # 4 contraction tiles
NPH = 2 * H        # 16 phases (head, q-half)

_cache = {}


def _build():
    nc = bacc.Bacc("TRN2")
    xbT = nc.declare_dram_parameter("xbT", [C, T], BF16, isOutput=False)
    wqT = nc.declare_dram_parameter("wqT", [C, C], BF16, isOutput=False)
    wkT = nc.declare_dram_parameter("wkT", [C, C], BF16, isOutput=False)
    wvT = nc.declare_dram_parameter("wvT", [C, C], BF16, isOutput=False)
    woT = nc.declare_dram_parameter("woT", [C, C], BF16, isOutput=False)
    # bias[:, 0, p] = bq slice, bias[:, 1, p] = bk slice
    bias = nc.declare_dram_parameter("bias", [128, 2, NP], F32, isOutput=False)
    bof = nc.declare_dram_parameter("bof", [1, C], F32R, isOutput=False)
    ones1 = nc.declare_dram_parameter("ones1", [1, 128], F32R, isOutput=False)
    out = nc.declare_dram_parameter("out", [TQ, C], F32, isOutput=True)

    with tile.TileContext(nc) as tc:
        with (
            tc.tile_pool(name="big", bufs=1) as bpool,
            tc.tile_pool(name="v2", bufs=2) as v2pool,
            tc.tile_pool(name="rc", bufs=4) as rpool,
            tc.tile_pool(name="ot", bufs=4) as opool,
            tc.tile_pool(name="ring", bufs=2, space="PSUM") as ring,
            tc.tile_pool(name="avp", bufs=1, space="PSUM") as avp,
            tc.tile_pool(name="prj", bufs=1, space="PSUM") as prj,
        ):
            # ---- static SBUF tiles ----
            xT = bpool.tile([128, CT, T], BF16, tag="xT")        # 32KB/part
            woTs = bpool.tile([128, CT, C], BF16, tag="woT")     # 4KB
            biasS = bpool.tile([128, 2, NP], F32, tag="bias")
            onesO = bpool.tile([65, 128], F32R, tag="ones")
            boS = bpool.tile([65, C], F32R, tag="bo")
            # fp8 K^T/Q^T, double-buffered by pair parity; [:,1,:] stays 0
            kf8a = bpool.tile([128, 2, T], F8, tag="kf8a")
            kf8b = bpool.tile([128, 2, T], F8, tag="kf8b")
            qf8a = bpool.tile([128, 2, TQ], F8, tag="qf8a")
            qf8b = bpool.tile([128, 2, TQ], F8, tag="qf8b")
            kf8 = [kf8a, kf8b]
            qf8 = [qf8a, qf8b]
            # exp output, double-buffered by phase parity
            at0 = bpool.tile([128, KT, 512], BF16, tag="at0")    # 32KB
            at1 = bpool.tile([128, KT, 512], BF16, tag="at1")    # 32KB
            at = [at0, at1]
            # normalized AV, [q, d-pair]; double-buffered by pair parity
            avn0 = bpool.tile([128, 8, 128], BF16, tag="avn0")
            avn1 = bpool.tile([128, 8, 128], BF16, tag="avn1")
            avn = [avn0, avn1]
            acat = bpool.tile([128, NP, TQ], BF16, tag="acat")   # 8KB
            oacc = bpool.tile([128, 8, C], F32, tag="oacc")      # 16KB
            wkS = bpool.tile([128, CT, C], BF16, tag="wkS")
            wqS = bpool.tile([128, CT, C], BF16, tag="wqS")
            wvS = bpool.tile([128, CT, C], BF16, tag="wvS")

            # ---- prologue DMAs ----
            xv = xbT.rearrange("(ct p) t -> p ct t", p=128)
            wkv = wkT.rearrange("(ct p) c -> p ct c", p=128)
            wqv = wqT.rearrange("(ct p) c -> p ct c", p=128)
            wvv = wvT.rearrange("(ct p) c -> p ct c", p=128)
            nc.sync.dma_start(wkS[:], wkv[:])
            nc.sync.dma_start(xT[:, :, 0:512], xv[:, :, 0:512])
            nc.sync.dma_start(wqS[:], wqv[:])
            nc.sync.dma_start(biasS[:], bias[:])
            nc.sync.dma_start(xT[:, :, 512:1024], xv[:, :, 512:1024])
            nc.sync.dma_start(wvS[:], wvv[:])
            for tch in range(1, 4):
                nc.sync.dma_start(
                    xT[:, :, tch * 1024:(tch + 1) * 1024],
                    xv[:, :, tch * 1024:(tch + 1) * 1024])
            nc.sync.dma_start(onesO[64:65, :],
                              ones1.rearrange("(o a) b -> o a b", o=1))
            nc.sync.dma_start(boS[64:65, :], bof.rearrange("(o a) b -> o a b", o=1))
            nc.sync.dma_start(woTs[:], woT.rearrange("(ct p) c -> p ct c", p=128))
            dz = bpool.tile([64, 2, 512], F8, tag="dz")
            nc.vector.memset(dz[:], 0.0)
            nc.vector.memset(kf8[0][:, 1, 0:1536], 0.0)
            nc.vector.memset(qf8[0][:, 1, :], 0.0)
            nc.gpsimd.memset(kf8[0][:, 1, 1536:T], 0.0)
            nc.gpsimd.memset(kf8[1][:, 1, :], 0.0)
            nc.gpsimd.memset(qf8[1][:, 1, :], 0.0)

            # ---- helper emitters (each returns a list of zero-arg thunks) --

            def k_piece(p, piece):
                # K^T cols piece*512:(piece+1)*512 -> kf8[p%2][:, 0, ...]
                pp = prj.tile([128, 512], F32, tag="prj")
                for ct in range(CT):
                    nc.tensor.matmul(
                        pp[:], wkS[:, ct, p * 128:(p + 1) * 128],
                        xT[:, ct, piece * 512:(piece + 1) * 512],
                        start=(ct == 0), stop=(ct == CT - 1))
                nc.vector.tensor_scalar_add(
                    kf8[p % 2][:, 0, piece * 512:(piece + 1) * 512], pp[:],
                    biasS[:, 1, p:p + 1])

            def q_piece(p, piece):
                pp = prj.tile([128, 512], F32, tag="prj")
                for ct in range(CT):
                    nc.tensor.matmul(
                        pp[:], wqS[:, ct, p * 128:(p + 1) * 128],
                        xT[:, ct, piece * 512:(piece + 1) * 512],
                        start=(ct == 0), stop=(ct == CT - 1))
                nc.vector.tensor_scalar_add(
                    qf8[p % 2][:, 0, piece * 512:(piece + 1) * 512], pp[:],
                    biasS[:, 0, p:p + 1])

            def v_piece(v2p, pg, j, pool=None):
                # V rows for k-tiles j, j+1 as separate exact-cover groups.
                for jj in range(2):
                    pv = (pool or prj).tile([128, 512], F32,
                                            tag="av" if pool is avp else "prj")
                    for ct in range(CT):
                        nc.tensor.matmul(
                            pv[:, 0:256],
                            xT[:, ct, (j + jj) * 128:(j + jj + 1) * 128],
                            wvS[:, ct, pg * 256:(pg + 1) * 256],
                            start=(ct == 0), stop=(ct == CT - 1))
                    nc.vector.tensor_copy(
                        v2p[:, j + jj, :, 0:64],
                        pv[:, 0:256].rearrange("p (h b) -> p h b", b=64))

            def new_v2p():
                v2p = v2pool.tile([128, KT, 4, 65], BF16, tag="v2p")
                nc.gpsimd.memset(v2p[:, :, :, 64], 1.0)
                return v2p

            def av_group(ph, qb, v2p, av_ap=None, pool=None):
                # AV for phase ph (= head h, q-half), query block qb (0..3)
                h, half = ph // 2, ph % 2
                hb = ph % 2  # at buffer parity
                d0 = (h % 2) * 64
                pb = (h // 2) % 2
                qbg = half * 4 + qb
                if av_ap is None:
                    pl = pool or avp
                    av_t = pl.tile([128, 512], F32,
                                   tag="av" if pl is avp else "prj")
                else:
                    av_t = av_ap
                for kt in range(KT):
                    nc.tensor.matmul(
                        av_t[:, 0:65],
                        at[hb][:, kt, qb * 128:(qb + 1) * 128],
                        v2p[:, kt, h % 4, :],
                        start=(kt == 0), stop=(kt == KT - 1))
                rec = rpool.tile([128, 1], F32, tag="rec")
                nc.vector.reciprocal_approx_fast(rec[:], av_t[:, 64:65])
                nc.vector.tensor_scalar(
                    avn[pb][:, qbg, d0:d0 + 64], av_t[:, 0:64],
                    rec[:], None, MULT)

            def o_piece(qt, po=None):
                if po is None:
                    po = prj.tile([128, 512], F32, tag="prj")
                nc.tensor.matmul(
                    po[:], acat[:, 3, qt * 128:(qt + 1) * 128],
                    woTs[:, 3, :], start=True, stop=True)
                ot = opool.tile([128, 512], F32, tag="ot")
                nc.vector.tensor_tensor(out=ot[:], in0=po[:],
                                        in1=oacc[:, qt, :], op=ADD)
                nc.sync.dma_start(out[qt * 128:(qt + 1) * 128, :], ot[:])

            # ---- prologue compute: pair-0 K/Q on ring slots (batched drains) --

            def ring_kq(groups):
                """groups: list of ('k'|'q', p, piece). One ring tile, one
                matmul group per bank, batched drains per contiguous run."""
                rt = ring.tile([128, 1536], F32, tag="ring")
                for g, (kind, p, piece) in enumerate(groups):
                    w = wkS if kind == "k" else wqS
                    for ct in range(CT):
                        nc.tensor.matmul(
                            rt[:, g * 512:(g + 1) * 512],
                            w[:, ct, p * 128:(p + 1) * 128],
                            xT[:, ct, piece * 512:(piece + 1) * 512],
                            start=(ct == 0), stop=(ct == CT - 1))
                # batched drains over contiguous same-kind runs
                g = 0
                while g < len(groups):
                    kind, p, piece = groups[g]
                    g2 = g
                    while (g2 + 1 < len(groups)
                           and groups[g2 + 1][0] == kind
                           and groups[g2 + 1][2] == groups[g2][2] + 1):
                        g2 += 1
                    dst = kf8[p % 2] if kind == "k" else qf8[p % 2]
                    bcol = 1 if kind == "k" else 0
                    nc.vector.tensor_scalar_add(
                        dst[:, 0, piece * 512:piece * 512 + (g2 - g + 1) * 512],
                        rt[:, g * 512:(g2 + 1) * 512],
                        biasS[:, bcol, p:p + 1])
                    g = g2 + 1

            # PE p-state warm-up on zeros while x loads
            wup = avp.tile([128, 512], F32, tag="av")
            for i in range(14):
                nc.tensor.matmul(wup[:], dz[:, :, 0:128], dz[:],
                                 start=True, stop=True, perf_mode=DR,
                                 tile_position=(0, 0))
            ring_kq([("k", 0, 0)])
            q_piece(0, 0)
            v2p_cur = new_v2p()

            # ---- main pipeline over 16 phases ----
            state = {"v2p": v2p_cur, "v2p_next": None, "pending": [],
                     "o_pending": []}

            def phase_background(ph):
                """Thunks to interleave into phase ph's chunk stream."""
                thunks = []
                h, half = ph // 2, ph % 2
                p = h // 2
                # deferred transposes/O-pieces from the previous phase first
                pend, state["pending"] = state["pending"], []
                thunks.extend(pend)
                # leftover pg0 V-pieces MUST precede phase-0's AV groups
                if ph == 1:
                    for j in range(28, KT, 2):
                        thunks.append(lambda j=j: v_piece(state["v2p"], 0, j))
                # AV of previous phase (+ deferred transpose & O-proj)
                if ph >= 1:
                    prev_h = (ph - 1) // 2
                    v2p_prev = (state["v2p_prev4"] if prev_h // 4 != h // 4
                                else state["v2p"])
                    # prj bank is idle on these phases: alternate AV banks
                    # to break the accumulator-WAR convoy
                    free_prj = ph in (8, 9, 12, 15)
                    for qb in range(4):
                        pl = prj if (free_prj and qb % 2) else None
                        thunks.append(lambda ph=ph, qb=qb, v=v2p_prev, pl=pl:
                                      av_tr_o(ph - 1, qb, v, pool=pl))
                # projection prep for pair p+1
                slot = ph % 4
                if p + 1 < NP:
                    if slot == 2:
                        for piece in range(5):
                            thunks.append(lambda p=p, piece=piece:
                                          k_piece(p + 1, piece))
                    elif slot == 3:
                        for piece in range(5, 8):
                            thunks.append(lambda p=p, piece=piece:
                                          k_piece(p + 1, piece))
                        for piece in range(2):
                            thunks.append(lambda p=p, piece=piece:
                                          q_piece(p + 1, piece))
                # pair-0 K piece 7 early in phase 0
                if ph == 0:
                    thunks.append(lambda: ring_kq(
                        [("q", 0, 1), ("k", 0, 1), ("k", 0, 2)]))
                    thunks.append(lambda: ring_kq(
                        [("k", 0, 3), ("k", 0, 4), ("k", 0, 5)]))
                    thunks.append(lambda: k_piece(0, 6))
                    thunks.append(lambda: k_piece(0, 7))
                    for i, j in enumerate(range(0, 28, 2)):
                        thunks.append(lambda j=j, i=i: v_piece(
                            state["v2p"], 0, j,
                            pool=avp if i % 2 else prj))
                if 4 <= ph <= 7:
                    if ph == 4:
                        def mkv():
                            state["v2p_next"] = new_v2p()
                        thunks.append(mkv)
                    for j in range((ph - 4) * 8, (ph - 4) * 8 + 8, 2):
                        thunks.append(lambda j=j: v_piece(state["v2p_next"],
                                                          1, j))
                if ph == 13:
                    for qt in range(4):
                        thunks.append(lambda qt=qt: o_partial(qt))
                if ph == 14:
                    for qt in range(4, 8):
                        thunks.append(lambda qt=qt: o_partial(qt))
                return thunks

            def o_partial(qt):
                po = prj.tile([128, 512], F32, tag="prj")
                for r in range(3):
                    nc.tensor.matmul(
                        po[:], acat[:, r, qt * 128:(qt + 1) * 128],
                        woTs[:, r, :], start=(r == 0), stop=False)
                nc.tensor.matmul(po[:], onesO[64:65, :], boS[64:65, :],
                                 start=False, stop=True)
                nc.vector.tensor_copy(oacc[:, qt, :], po[:])

            def tr_o(p, qbg, po=None):
                nc.sync.dma_start_transpose(
                    acat[:, p, qbg * 128:(qbg + 1) * 128],
                    avn[p % 2][:, qbg, :])
                if p == NP - 1:
                    o_piece(qbg, po)

            def av_tr_o(ph, qb, v2p, av_ap=None, po=None, defer=True,
                        pool=None):
                """AV group + (for odd heads) transpose + (pair 3) O-proj."""
                av_group(ph, qb, v2p, av_ap, pool=pool)
                h, half = ph // 2, ph % 2
                if h % 2 == 1:
                    p = h // 2
                    qbg = half * 4 + qb
                    if p == NP - 1:
                        # last pair: transpose deferred, O-piece to epilogue
                        state["pending"].append(
                            lambda qbg=qbg: nc.sync.dma_start_transpose(
                                acat[:, 3, qbg * 128:(qbg + 1) * 128],
                                avn[1][:, qbg, :]))
                        state["o_pending"].append(qbg)
                    elif defer:
                        state["pending"].append(
                            lambda p=p, qbg=qbg, po=po: tr_o(p, qbg, po))
                    else:
                        tr_o(p, qbg, po)

            for ph in range(NPH):
                h, half = ph // 2, ph % 2
                if ph == 8:
                    state["v2p_prev4"] = state["v2p"]
                    state["v2p"] = state["v2p_next"]
                state.setdefault("v2p_prev4", state["v2p"])
                d0 = (h % 2) * 64
                kcur, qcur = kf8[h // 2 % 2], qf8[h // 2 % 2]
                bg = phase_background(ph)
                bgi = 0
                # 11 chunks: 10x3 kt + 1x2 kt
                for c in range(11):
                    n = 3 if c < 10 else 2
                    ring_t = ring.tile([128, 1536], F32, tag="ring")
                    for jj in range(n):
                        kt = 3 * c + jj
                        nc.tensor.matmul(
                            ring_t[:, jj * 512:(jj + 1) * 512],
                            kcur[d0:d0 + 64, :, kt * 128:(kt + 1) * 128],
                            qcur[d0:d0 + 64, :, half * 512:(half + 1) * 512],
                            start=True, stop=True, perf_mode=DR,
                            tile_position=(d0, 0))
                    nc.scalar.activation(
                        at[ph % 2][:, 3 * c:3 * c + n, :],
                        ring_t[:, 0:512 * n].rearrange("p (a b) -> p a b", b=512),
                        AF.Exp, scale=0.125)
                    # interleave background work: spread across chunks
                    n_bg = (len(bg) * (c + 1)) // 11 - (len(bg) * c) // 11
                    for _ in range(n_bg):
                        bg[bgi]()
                        bgi += 1
                assert bgi == len(bg)
                if ph == NPH - 1:
                    # flush pair-3 transposes first, then their O-pieces
                    pend, state["pending"] = state["pending"], []
                    for th in pend:
                        th()
                    for qt in state["o_pending"][:4]:
                        o_piece(qt)
                    state["o_pending"] = state["o_pending"][4:]

            # ---- epilogue: AV of phase 15 + pair-3 transposes + O-proj ----
            # ring banks are free: give every AV group and O-piece its own
            # bank and software-pipeline so no PE wait blocks later work.
            for th in state["pending"]:
                th()
            h15 = (NPH - 1) // 2
            v2p15 = state["v2p"]
            avA = avp.tile([128, 512], F32, tag="av")
            epA = ring.tile([128, 1536], F32, tag="ring")
            av_aps = [avA, epA[:, 0:512], epA[:, 512:1024], epA[:, 1024:1536]]
            for kt in range(KT):
                for qb in range(4):
                    nc.tensor.matmul(
                        av_aps[qb][:, 0:65],
                        at[(NPH - 1) % 2][:, kt, qb * 128:(qb + 1) * 128],
                        v2p15[:, kt, h15 % 4, :],
                        start=(kt == 0), stop=(kt == KT - 1))

            def norm_tr(qb, t):
                qbg = 4 + qb
                rec = rpool.tile([128, 1], F32, tag="rec")
                nc.vector.reciprocal_approx_fast(rec[:], t[:, 64:65])
                nc.vector.tensor_scalar(
                    avn[1][:, qbg, 64:128], t[:, 0:64], rec[:], None, MULT)
                nc.sync.dma_start_transpose(
                    acat[:, 3, qbg * 128:(qbg + 1) * 128],
                    avn[1][:, qbg, :])

            for qb in range(4):
                norm_tr(qb, av_aps[qb])
            epB = ring.tile([128, 1536], F32, tag="ring")
            avB = avp.tile([128, 512], F32, tag="av")
            for i, qt in enumerate([4, 5, 6, 7]):
                o_piece(qt, po=epB[:, i * 512:(i + 1) * 512]
                        if i < 3 else avB)

    nc.compile()
    return nc


def _prep_inputs(x, Wq, bq, Wk, bk, Wv, bv, Wo, bo):
    bf = ml_dtypes.bfloat16
    wqT = np.ascontiguousarray(Wq.T).astype(bf)
    wkT = np.ascontiguousarray(Wk.T).astype(bf)
    wvT = np.ascontiguousarray(Wv.T).astype(bf)
    woT = np.ascontiguousarray(Wo.T).astype(bf)
    bias = np.stack([
        bq.reshape(NP, 128).T,
        bk.reshape(NP, 128).T,
    ], axis=1).astype(np.float32)          # [128, 2, NP]
    bias = np.ascontiguousarray(bias)
    bof = np.ascontiguousarray(
        (bo.astype(np.float64) + bv.astype(np.float64) @ Wo.astype(np.float64).T)
        .reshape(1, C)).astype(np.float32)
    ones1 = np.ones((1, 128), np.float32)
    in_maps = []
    for i in range(8):
        b, q0 = i // 4, (i % 4) * TQ
        xbT = np.ascontiguousarray(np.roll(x[b].T, -q0, axis=1)).astype(bf)
        in_maps.append({
            "xbT": xbT, "wqT": wqT, "wkT": wkT, "wvT": wvT, "woT": woT,
            "bias": bias, "bof": bof, "ones1": ones1,
        })
    return in_maps


def kernel(x, Wq, bq, Wk, bk, Wv, bv, Wo, bo):
    x = np.asarray(x, np.float32)
    args = [np.asarray(a, np.float32) for a in
            (Wq, bq, Wk, bk, Wv, bv, Wo, bo)]
    if "nc" not in _cache:
        _cache["nc"] = _build()
    nc = _cache["nc"]
    in_maps = _prep_inputs(x, *args)
    res = run_bass_kernel_spmd(nc, in_maps, list(range(8)))
    outf = np.empty((B, T, C), np.float32)
    for i in range(8):
        b, q0 = i // 4, (i % 4) * TQ
        outf[b, q0:q0 + TQ, :] = res.results[i]["out"]
    return outf


# revision 45
# speedup vs baseline: 1.0202x; 1.0072x over previous
"""Multi-head attention (B=2, T=4096, D=512, H=8) on 8 Trainium2 cores.

Sharding: core i handles batch b=i//4, query rows q0=(i%4)*1024 .. q0+1024,
all 8 heads (full K/V of its batch computed on-core; no collectives).
Host pre-transposes x and weights (bf16) and rolls x along T per core so
each core's query block sits at columns 0:1024.

v2 pipeline (exp-bound design):
- Projections in bf16 (x, W all bf16; f32 PSUM accumulation).
- Scores via fp8e4(e4m3) DoubleRow matmuls: K^T/Q^T drained to fp8 with a
  zeroed second k-tile slot, so one DR matmul contracts the full d_k=64 at
  0.5 cycles/row.  exp applies the 1/sqrt(d_k)=1/8 scale for free.
- exp on the Activation engine in 3-bank (1536-col) PSUM chunks, double
  buffered through a 6-bank ring; output bf16 `at` tiles.
- AV with swapped operands (stationary=at [128k,128q], moving=V|1 [128k,65])
  accumulating [128q, 65] in a single PSUM bank; per-partition rowsum ->
  reciprocal_approx_fast + normalize to bf16 (DVE), then DMA-transpose
  [128q,128d] -> acat [128d, q].  Output projection partials (pairs 0-2)
  pre-accumulated into SBUF so the tail is one matmul + add per block.
- bv is folded into bo on the host (bo' = bo + bv @ Wo.T), bq/bk folded
  into the fp8 drains.
- Output projection in bf16 + f32r bias matmul, f32 result.
"""
import sys
sys.path.insert(0, "/opt/trn_rl_repo")

import numpy as np
import ml_dtypes
import concourse.bacc as bacc
import concourse.mybir as mybir
import concourse.tile as tile
from concourse.bass_utils import run_bass_kernel_spmd

F32 = mybir.dt.float32
F32R = mybir.dt.float32r
BF16 = mybir.dt.bfloat16
F8 = mybir.dt.float8e4
AF = mybir.ActivationFunctionType
ADD = mybir.AluOpType.add
MULT = mybir.AluOpType.mult
DR = mybir.MatmulPerfMode.DoubleRow

B, T, C = 2, 4096, 512
H, DK = 8, 64
TQ = 1024          # queries per core
NP = 4             # head pairs
KT = T // 128      # 32 k-tiles
CT = C // 128      # 4 contraction tiles
NPH = 2 * H        # 16 phases (head, q-half)

_cache = {}


def _build():
    nc = bacc.Bacc("TRN2")
    xbT = nc.declare_dram_parameter("xbT", [C, T], BF16, isOutput=False)
    wqT = nc.declare_dram_parameter("wqT", [C, C], BF16, isOutput=False)
    wkT = nc.declare_dram_parameter("wkT", [C, C], BF16, isOutput=False)
    wvT = nc.declare_dram_parameter("wvT", [C, C], BF16, isOutput=False)
    woT = nc.declare_dram_parameter("woT", [C, C], BF16, isOutput=False)
    # bias[:, 0, p] = bq slice, bias[:, 1, p] = bk slice
    bias = nc.declare_dram_parameter("bias", [128, 2, NP], F32, isOutput=False)
    bof = nc.declare_dram_parameter("bof", [1, C], F32R, isOutput=False)
    ones1 = nc.declare_dram_parameter("ones1", [1, 128], F32R, isOutput=False)
    out = nc.declare_dram_parameter("out", [TQ, C], BF16, isOutput=True)

    with tile.TileContext(nc) as tc:
        with (
            tc.tile_pool(name="big", bufs=1) as bpool,
            tc.tile_pool(name="v2", bufs=2) as v2pool,
            tc.tile_pool(name="rc", bufs=4) as rpool,
            tc.tile_pool(name="ot", bufs=4) as opool,
            tc.tile_pool(name="ring", bufs=2, space="PSUM") as ring,
            tc.tile_pool(name="avp", bufs=1, space="PSUM") as avp,
            tc.tile_pool(name="prj", bufs=1, space="PSUM") as prj,
        ):
            # ---- static SBUF tiles ----
            xT = bpool.tile([128, CT, T], BF16, tag="xT")        # 32KB/part
            woTs = bpool.tile([128, CT, C], BF16, tag="woT")     # 4KB
            biasS = bpool.tile([128, 2, NP], F32, tag="bias")
            onesO = bpool.tile([65, 128], F32R, tag="ones")
            boS = bpool.tile([65, C], F32R, tag="bo")
            # fp8 K^T/Q^T, double-buffered by pair parity; [:,1,:] stays 0
            kf8a = bpool.tile([128, 2, T], F8, tag="kf8a")
            kf8b = bpool.tile([128, 2, T], F8, tag="kf8b")
            qf8a = bpool.tile([128, 2, TQ], F8, tag="qf8a")
            qf8b = bpool.tile([128, 2, TQ], F8, tag="qf8b")
            kf8 = [kf8a, kf8b]
            qf8 = [qf8a, qf8b]
            # exp output, double-buffered by phase parity
            at0 = bpool.tile([128, KT, 512], BF16, tag="at0")    # 32KB
            at1 = bpool.tile([128, KT, 512], BF16, tag="at1")    # 32KB
            at = [at0, at1]
            # normalized AV, [q, d-pair]; double-buffered by pair parity
            avn0 = bpool.tile([128, 8, 128], BF16, tag="avn0")
            avn1 = bpool.tile([128, 8, 128], BF16, tag="avn1")
            avn = [avn0, avn1]
            acat = bpool.tile([128, NP, TQ], BF16, tag="acat")   # 8KB
            oacc = bpool.tile([128, 8, C], F32, tag="oacc")      # 16KB
            wkS = bpool.tile([128, CT, C], BF16, tag="wkS")
            wqS = bpool.tile([128, CT, C], BF16, tag="wqS")
            wvS = bpool.tile([128, CT, C], BF16, tag="wvS")

            # ---- prologue DMAs ----
            xv = xbT.rearrange("(ct p) t -> p ct t", p=128)
            wkv = wkT.rearrange("(ct p) c -> p ct c", p=128)
            wqv = wqT.rearrange("(ct p) c -> p ct c", p=128)
            wvv = wvT.rearrange("(ct p) c -> p ct c", p=128)
            nc.sync.dma_start(wkS[:], wkv[:])
            nc.sync.dma_start(xT[:, :, 0:512], xv[:, :, 0:512])
            nc.sync.dma_start(wqS[:], wqv[:])
            nc.sync.dma_start(biasS[:], bias[:])
            nc.sync.dma_start(xT[:, :, 512:1024], xv[:, :, 512:1024])
            nc.sync.dma_start(wvS[:], wvv[:])
            for tch in range(1, 4):
                nc.sync.dma_start(
                    xT[:, :, tch * 1024:(tch + 1) * 1024],
                    xv[:, :, tch * 1024:(tch + 1) * 1024])
            nc.sync.dma_start(onesO[64:65, :],
                              ones1.rearrange("(o a) b -> o a b", o=1))
            nc.sync.dma_start(boS[64:65, :], bof.rearrange("(o a) b -> o a b", o=1))
            nc.sync.dma_start(woTs[:], woT.rearrange("(ct p) c -> p ct c", p=128))
            dz = bpool.tile([64, 2, 512], F8, tag="dz")
            nc.vector.memset(dz[:], 0.0)
            nc.vector.memset(kf8[0][:, 1, 0:1536], 0.0)
            nc.vector.memset(qf8[0][:, 1, :], 0.0)
            nc.gpsimd.memset(kf8[0][:, 1, 1536:T], 0.0)
            nc.gpsimd.memset(kf8[1][:, 1, :], 0.0)
            nc.gpsimd.memset(qf8[1][:, 1, :], 0.0)

            # ---- helper emitters (each returns a list of zero-arg thunks) --

            def k_piece(p, piece):
                # K^T cols piece*512:(piece+1)*512 -> kf8[p%2][:, 0, ...]
                pp = prj.tile([128, 512], F32, tag="prj")
                for ct in range(CT):
                    nc.tensor.matmul(
                        pp[:], wkS[:, ct, p * 128:(p + 1) * 128],
                        xT[:, ct, piece * 512:(piece + 1) * 512],
                        start=(ct == 0), stop=(ct == CT - 1))
                nc.vector.tensor_scalar_add(
                    kf8[p % 2][:, 0, piece * 512:(piece + 1) * 512], pp[:],
                    biasS[:, 1, p:p + 1])

            def q_piece(p, piece):
                pp = prj.tile([128, 512], F32, tag="prj")
                for ct in range(CT):
                    nc.tensor.matmul(
                        pp[:], wqS[:, ct, p * 128:(p + 1) * 128],
                        xT[:, ct, piece * 512:(piece + 1) * 512],
                        start=(ct == 0), stop=(ct == CT - 1))
                nc.vector.tensor_scalar_add(
                    qf8[p % 2][:, 0, piece * 512:(piece + 1) * 512], pp[:],
                    biasS[:, 0, p:p + 1])

            def v_piece(v2p, pg, j, pool=None):
                # V rows for k-tiles j, j+1 as separate exact-cover groups.
                for jj in range(2):
                    pv = (pool or prj).tile([128, 512], F32,
                                            tag="av" if pool is avp else "prj")
                    for ct in range(CT):
                        nc.tensor.matmul(
                            pv[:, 0:256],
                            xT[:, ct, (j + jj) * 128:(j + jj + 1) * 128],
                            wvS[:, ct, pg * 256:(pg + 1) * 256],
                            start=(ct == 0), stop=(ct == CT - 1))
                    nc.vector.tensor_copy(
                        v2p[:, j + jj, :, 0:64],
                        pv[:, 0:256].rearrange("p (h b) -> p h b", b=64))

            def new_v2p():
                v2p = v2pool.tile([128, KT, 4, 65], BF16, tag="v2p")
                nc.gpsimd.memset(v2p[:, :, :, 64], 1.0)
                return v2p

            def av_group(ph, qb, v2p, av_ap=None, pool=None):
                # AV for phase ph (= head h, q-half), query block qb (0..3)
                h, half = ph // 2, ph % 2
                hb = ph % 2  # at buffer parity
                d0 = (h % 2) * 64
                pb = (h // 2) % 2
                qbg = half * 4 + qb
                if av_ap is None:
                    pl = pool or avp
                    av_t = pl.tile([128, 512], F32,
                                   tag="av" if pl is avp else "prj")
                else:
                    av_t = av_ap
                for kt in range(KT):
                    nc.tensor.matmul(
                        av_t[:, 0:65],
                        at[hb][:, kt, qb * 128:(qb + 1) * 128],
                        v2p[:, kt, h % 4, :],
                        start=(kt == 0), stop=(kt == KT - 1))
                rec = rpool.tile([128, 1], F32, tag="rec")
                nc.vector.reciprocal_approx_fast(rec[:], av_t[:, 64:65])
                nc.vector.tensor_scalar(
                    avn[pb][:, qbg, d0:d0 + 64], av_t[:, 0:64],
                    rec[:], None, MULT)

            def o_piece(qt, po=None):
                if po is None:
                    po = prj.tile([128, 512], F32, tag="prj")
                nc.tensor.matmul(
                    po[:], acat[:, 3, qt * 128:(qt + 1) * 128],
                    woTs[:, 3, :], start=True, stop=True)
                ot = opool.tile([128, 512], BF16, tag="ot")
                nc.vector.tensor_tensor(out=ot[:], in0=po[:],
                                        in1=oacc[:, qt, :], op=ADD)
                nc.sync.dma_start(out[qt * 128:(qt + 1) * 128, :], ot[:])

            # ---- prologue compute: pair-0 K/Q on ring slots (batched drains) --

            def ring_kq(groups):
                """groups: list of ('k'|'q', p, piece). One ring tile, one
                matmul group per bank, batched drains per contiguous run."""
                rt = ring.tile([128, 1536], F32, tag="ring")
                for g, (kind, p, piece) in enumerate(groups):
                    w = wkS if kind == "k" else wqS
                    for ct in range(CT):
                        nc.tensor.matmul(
                            rt[:, g * 512:(g + 1) * 512],
                            w[:, ct, p * 128:(p + 1) * 128],
                            xT[:, ct, piece * 512:(piece + 1) * 512],
                            start=(ct == 0), stop=(ct == CT - 1))
                # batched drains over contiguous same-kind runs
                g = 0
                while g < len(groups):
                    kind, p, piece = groups[g]
                    g2 = g
                    while (g2 + 1 < len(groups)
                           and groups[g2 + 1][0] == kind
                           and groups[g2 + 1][2] == groups[g2][2] + 1):
                        g2 += 1
                    dst = kf8[p % 2] if kind == "k" else qf8[p % 2]
                    bcol = 1 if kind == "k" else 0
                    nc.vector.tensor_scalar_add(
                        dst[:, 0, piece * 512:piece * 512 + (g2 - g + 1) * 512],
                        rt[:, g * 512:(g2 + 1) * 512],
                        biasS[:, bcol, p:p + 1])
                    g = g2 + 1

            # PE p-state warm-up on zeros while x loads
            wup = avp.tile([128, 512], F32, tag="av")
            for i in range(14):
                nc.tensor.matmul(wup[:], dz[:, :, 0:128], dz[:],
                                 start=True, stop=True, perf_mode=DR,
                                 tile_position=(0, 0))
            ring_kq([("k", 0, 0)])
            q_piece(0, 0)
            v2p_cur = new_v2p()

            # ---- main pipeline over 16 phases ----
            state = {"v2p": v2p_cur, "v2p_next": None, "pending": [],
                     "o_pending": []}

            def phase_background(ph):
                """Thunks to interleave into phase ph's chunk stream."""
                thunks = []
                h, half = ph // 2, ph % 2
                p = h // 2
                # deferred transposes/O-pieces from the previous phase first
                pend, state["pending"] = state["pending"], []
                thunks.extend(pend)
                # leftover pg0 V-pieces MUST precede phase-0's AV groups
                if ph == 1:
                    for j in range(28, KT, 2):
                        thunks.append(lambda j=j: v_piece(state["v2p"], 0, j))
                # AV of previous phase (+ deferred transpose & O-proj)
                if ph >= 1:
                    prev_h = (ph - 1) // 2
                    v2p_prev = (state["v2p_prev4"] if prev_h // 4 != h // 4
                                else state["v2p"])
                    # prj bank is idle on these phases: alternate AV banks
                    # to break the accumulator-WAR convoy
                    free_prj = ph in (8, 9, 12, 15)
                    for qb in range(4):
                        pl = prj if (free_prj and qb % 2) else None
                        thunks.append(lambda ph=ph, qb=qb, v=v2p_prev, pl=pl:
                                      av_tr_o(ph - 1, qb, v, pool=pl))
                # projection prep for pair p+1
                slot = ph % 4
                if p + 1 < NP:
                    if slot == 2:
                        for piece in range(5):
                            thunks.append(lambda p=p, piece=piece:
                                          k_piece(p + 1, piece))
                    elif slot == 3:
                        for piece in range(5, 8):
                            thunks.append(lambda p=p, piece=piece:
                                          k_piece(p + 1, piece))
                        for piece in range(2):
                            thunks.append(lambda p=p, piece=piece:
                                          q_piece(p + 1, piece))
                # pair-0 K piece 7 early in phase 0
                if ph == 0:
                    thunks.append(lambda: ring_kq([("q", 0, 1), ("k", 0, 1)]))
                    thunks.append(lambda: ring_kq([("k", 0, 2), ("k", 0, 3)]))
                    thunks.append(lambda: ring_kq([("k", 0, 4), ("k", 0, 5)]))
                    thunks.append(lambda: k_piece(0, 6))
                    thunks.append(lambda: k_piece(0, 7))
                    for i, j in enumerate(range(0, 28, 2)):
                        thunks.append(lambda j=j, i=i: v_piece(
                            state["v2p"], 0, j,
                            pool=avp if i % 2 else prj))
                if 4 <= ph <= 7:
                    if ph == 4:
                        def mkv():
                            state["v2p_next"] = new_v2p()
                        thunks.append(mkv)
                    for j in range((ph - 4) * 8, (ph - 4) * 8 + 8, 2):
                        thunks.append(lambda j=j: v_piece(state["v2p_next"],
                                                          1, j))
                if ph == 13:
                    for qt in range(4):
                        thunks.append(lambda qt=qt: o_partial(qt))
                if ph == 14:
                    for qt in range(4, 8):
                        thunks.append(lambda qt=qt: o_partial(qt))
                return thunks

            def o_partial(qt):
                po = prj.tile([128, 512], F32, tag="prj")
                for r in range(3):
                    nc.tensor.matmul(
                        po[:], acat[:, r, qt * 128:(qt + 1) * 128],
                        woTs[:, r, :], start=(r == 0), stop=False)
                nc.tensor.matmul(po[:], onesO[64:65, :], boS[64:65, :],
                                 start=False, stop=True)
                nc.vector.tensor_copy(oacc[:, qt, :], po[:])

            def tr_o(p, qbg, po=None):
                nc.sync.dma_start_transpose(
                    acat[:, p, qbg * 128:(qbg + 1) * 128],
                    avn[p % 2][:, qbg, :])
                if p == NP - 1:
                    o_piece(qbg, po)

            def av_tr_o(ph, qb, v2p, av_ap=None, po=None, defer=True,
                        pool=None):
                """AV group + (for odd heads) transpose + (pair 3) O-proj."""
                av_group(ph, qb, v2p, av_ap, pool=pool)
                h, half = ph // 2, ph % 2
                if h % 2 == 1:
                    p = h // 2
                    qbg = half * 4 + qb
                    if p == NP - 1:
                        # last pair: transpose deferred, O-piece to epilogue
                        state["pending"].append(
                            lambda qbg=qbg: nc.sync.dma_start_transpose(
                                acat[:, 3, qbg * 128:(qbg + 1) * 128],
                                avn[1][:, qbg, :]))
                        state["o_pending"].append(qbg)
                    elif defer:
                        state["pending"].append(
                            lambda p=p, qbg=qbg, po=po: tr_o(p, qbg, po))
                    else:
                        tr_o(p, qbg, po)

            for ph in range(NPH):
                h, half = ph // 2, ph % 2
                if ph == 8:
                    state["v2p_prev4"] = state["v2p"]
                    state["v2p"] = state["v2p_next"]
                state.setdefault("v2p_prev4", state["v2p"])
                d0 = (h % 2) * 64
                kcur, qcur = kf8[h // 2 % 2], qf8[h // 2 % 2]
                bg = phase_background(ph)
                bgi = 0
                # 11 chunks: 10x3 kt + 1x2 kt
                for c in range(11):
                    n = 3 if c < 10 else 2
                    ring_t = ring.tile([128, 1536], F32, tag="ring")
                    for jj in range(n):
                        kt = 3 * c + jj
                        nc.tensor.matmul(
                            ring_t[:, jj * 512:(jj + 1) * 512],
                            kcur[d0:d0 + 64, :, kt * 128:(kt + 1) * 128],
                            qcur[d0:d0 + 64, :, half * 512:(half + 1) * 512],
                            start=True, stop=True, perf_mode=DR,
                            tile_position=(d0, 0))
                    nc.scalar.activation(
                        at[ph % 2][:, 3 * c:3 * c + n, :],
                        ring_t[:, 0:512 * n].rearrange("p (a b) -> p a b", b=512),
                        AF.Exp, scale=0.125)
                    # interleave background work: spread across chunks
                    n_bg = (len(bg) * (c + 1)) // 11 - (len(bg) * c) // 11
                    for _ in range(n_bg):
                        bg[bgi]()
                        bgi += 1
                assert bgi == len(bg)
                if ph == NPH - 1:
                    # flush pair-3 transposes first, then their O-pieces
                    pend, state["pending"] = state["pending"], []
                    for th in pend:
                        th()
                    for qt in state["o_pending"][:4]:
                        o_piece(qt)
                    state["o_pending"] = state["o_pending"][4:]

            # ---- epilogue: AV of phase 15 + pair-3 transposes + O-proj ----
            # ring banks are free: give every AV group and O-piece its own
            # bank and software-pipeline so no PE wait blocks later work.
            for th in state["pending"]:
                th()
            h15 = (NPH - 1) // 2
            v2p15 = state["v2p"]
            avA = avp.tile([128, 512], F32, tag="av")
            epA = ring.tile([128, 1536], F32, tag="ring")
            av_aps = [avA, epA[:, 0:512], epA[:, 512:1024], epA[:, 1024:1536]]
            for kt in range(KT):
                for qb in range(4):
                    nc.tensor.matmul(
                        av_aps[qb][:, 0:65],
                        at[(NPH - 1) % 2][:, kt, qb * 128:(qb + 1) * 128],
                        v2p15[:, kt, h15 % 4, :],
                        start=(kt == 0), stop=(kt == KT - 1))

            def norm_tr(qb, t):
                qbg = 4 + qb
                rec = rpool.tile([128, 1], F32, tag="rec")
                nc.vector.reciprocal_approx_fast(rec[:], t[:, 64:65])
                nc.vector.tensor_scalar(
                    avn[1][:, qbg, 64:128], t[:, 0:64], rec[:], None, MULT)
                nc.sync.dma_start_transpose(
                    acat[:, 3, qbg * 128:(qbg + 1) * 128],
                    avn[1][:, qbg, :])

            for qb in range(4):
                norm_tr(qb, av_aps[qb])
            epB = ring.tile([128, 1536], F32, tag="ring")
            avB = avp.tile([128, 512], F32, tag="av")
            for i, qt in enumerate([4, 5, 6, 7]):
                o_piece(qt, po=epB[:, i * 512:(i + 1) * 512]
                        if i < 3 else avB)

    nc.compile()
    return nc


def _prep_inputs(x, Wq, bq, Wk, bk, Wv, bv, Wo, bo):
    bf = ml_dtypes.bfloat16
    wqT = np.ascontiguousarray(Wq.T).astype(bf)
    wkT = np.ascontiguousarray(Wk.T).astype(bf)
    wvT = np.ascontiguousarray(Wv.T).astype(bf)
    woT = np.ascontiguousarray(Wo.T).astype(bf)
    bias = np.stack([
        bq.reshape(NP, 128).T,
        bk.reshape(NP, 128).T,
    ], axis=1).astype(np.float32)          # [128, 2, NP]
    bias = np.ascontiguousarray(bias)
    bof = np.ascontiguousarray(
        (bo.astype(np.float64) + bv.astype(np.float64) @ Wo.astype(np.float64).T)
        .reshape(1, C)).astype(np.float32)
    ones1 = np.ones((1, 128), np.float32)
    in_maps = []
    for i in range(8):
        b, q0 = i // 4, (i % 4) * TQ
        xbT = np.ascontiguousarray(np.roll(x[b].T, -q0, axis=1)).astype(bf)
        in_maps.append({
            "xbT": xbT, "wqT": wqT, "wkT": wkT, "wvT": wvT, "woT": woT,
            "bias": bias, "bof": bof, "ones1": ones1,
        })
    return in_maps


def kernel(x, Wq, bq, Wk, bk, Wv, bv, Wo, bo):
    x = np.asarray(x, np.float32)
    args = [np.asarray(a, np.float32) for a in
            (Wq, bq, Wk, bk, Wv, bv, Wo, bo)]
    if "nc" not in _cache:
        _cache["nc"] = _build()
    nc = _cache["nc"]
    in_maps = _prep_inputs(x, *args)
    res = run_bass_kernel_spmd(nc, in_maps, list(range(8)))
    outf = np.empty((B, T, C), np.float32)
    for i in range(8):
        b, q0 = i // 4, (i % 4) * TQ
        outf[b, q0:q0 + TQ, :] = res.results[i]["out"].astype(np.float32)
    return outf
